# revision 2
# baseline (speedup 1.0000x reference)
"""Trainium2 Bass kernel for nn_IntraAgg (GNN mean-neighbor aggregation).

reference:
    valid[b,k] = k < neigh_counts[b]
    out = relu( (sum_k valid[b,k] * features[neigh_idx[b,k]]) / neigh_counts[b] )

Strategy (8 NeuronCores, data-parallel over the batch; two-pass SWDGE gather):
  The old per-128-row indirect-DMA gather was bottlenecked by the ~1.1 us
  fixed SWDGE emission cost per InstDMACopy (151 instructions/core -> 174 us
  serial on the GpSimd queue).  dma_gather (InstDMAGatherAnt, mlp ucode
  library) moves thousands of rows per instruction, but its indices are
  int16, so a 1M-row table needs segmentation:

  - phase 1: for each of 31 table segments of 32768 rows, one dma_gather
    pulls that segment's unique needed rows into an SBUF staging tile
    (row j of segment s lands at partition j%128, chunk s*cs + j//128).
  - phase 1.5: two big HWDGE DMAs copy staging to a contiguous DRAM scratch
    table [R, 64] (R = 31*cs*128 + 128; last 128 rows are a zeroed dump
    block for padding slots).
  - phase 2: per 128-node block, one dma_gather from scratch with indices in
    slot-major order reproduces the [128, kj*64] per-block layout (invalid
    slots -> dump row = zeros), then the same DVE strided reduce + ACT
    relu(x * 1/count) + store as before.

  dma_scatter_add was measured to LOSE updates for duplicate target rows on
  hardware (pipelined RMW hazard), so the regroup goes through the scratch
  bounce instead.

  Nodes are count-sorted per core so block kj (gather width) shrinks across
  blocks; num_idxs > 1024 requires single_packet=False (64-desc packet cap).
"""

import numpy as np

N_NODES = 1_000_000
FEAT_DIM = 64
BATCH = 8192
MAX_NEIGH = 32
N_CORES = 8
BLK = 128
SEG = 32768                      # phase-1 segment rows (int16 index range)
NSEG = -(-N_NODES // SEG)        # 31

_KERNEL_CACHE = {}


def _split_multi_waits(nc):
    """walrus codegen accepts at most one sync-wait per instruction: hoist
    extra waits onto NoOp instructions inserted just before."""
    import bass_rust

    for fn in nc.m.functions:
        for bb in fn.blocks:
            new_list = []
            for inst in bb.instructions:
                si = inst.sync_info
                if si is not None and si.on_wait is not None and len(si.on_wait) > 1:
                    waits = list(si.on_wait)
                    for j, w in enumerate(waits[:-1]):
                        nop = bass_rust.InstNoOp(name=f"{inst.name}-sw{j}")
                        nop.engine = inst.engine
                        nop.sync_info = bass_rust.SyncInfo(on_wait=[w], on_update=[])
                        new_list.append(nop)
                    inst.sync_info = bass_rust.SyncInfo(
                        on_wait=[waits[-1]], on_update=list(si.on_update or [])
                    )
                new_list.append(inst)
            bb.instructions = new_list


def _wrap16(arr, width):
    """int16 index list -> [128, width/16] SWDGE layout (entry i at
    [i%16, i//16], 16-partition pattern replicated 8x vertically)."""
    a = np.asarray(arr, dtype=np.int16)
    assert a.size % 16 == 0 and a.size // 16 <= width
    buf = np.full((16, width), -1, dtype=np.int16)
    w = a.reshape(-1, 16).T          # [16, n/16]
    buf[:, : w.shape[1]] = w
    return np.tile(buf, (8, 1))


def build_nc(m_cap, k_sched, n_nodes=N_NODES, b_loc=BATCH // N_CORES,
             d=FEAT_DIM):
    """Per-core SPMD program.  m_cap: uniform padded index count per phase-1
    segment gather; k_sched: per-block neighbor-slot counts (count-sorted)."""
    from concourse import bass, mybir, bass_isa
    from concourse.tile import TileContext
    from concourse.library_config import mlp

    nblk = b_loc // BLK
    assert len(k_sched) == nblk
    cs = -(-m_cap // 128)            # staging chunks per segment
    C = NSEG * cs                    # total staging chunks
    R = C * BLK + BLK                # scratch rows (+ zeroed dump block)
    dump = C * BLK
    assert R <= 32767, R
    c1 = NSEG * (m_cap // 16)        # i1 columns
    c2 = sum(kj * 8 for kj in k_sched)

    nc = bass.Bass()
    feat = nc.declare_dram_parameter("feat", [n_nodes, d], mybir.dt.float32,
                                     isOutput=False)
    i1 = nc.declare_dram_parameter("i1", [128, c1], mybir.dt.int16,
                                   isOutput=False)
    i2 = nc.declare_dram_parameter("i2", [128, c2], mybir.dt.int16,
                                   isOutput=False)
    recip = nc.declare_dram_parameter("recip", [BLK, nblk], mybir.dt.float32,
                                      isOutput=False)
    scr = nc.declare_dram_parameter("scr", [R, d], mybir.dt.float32,
                                    isOutput=True)
    out = nc.declare_dram_parameter("out", [b_loc, d], mybir.dt.float32,
                                    isOutput=True)

    fp32 = mybir.dt.float32
    i16 = mybir.dt.int16
    with TileContext(nc) as tc:
        nc.gpsimd.add_instruction(bass_isa.InstPseudoReloadLibraryIndex(
            name=f"I-{nc.next_id()}", ins=[], outs=[], lib_index=mlp.index))
        with tc.tile_pool(name="const", bufs=1) as constp, \
             tc.tile_pool(name="stag", bufs=1) as stag, \
             tc.tile_pool(name="gp", bufs=3) as gp, \
             tc.tile_pool(name="redp", bufs=3) as redp, \
             tc.tile_pool(name="outp", bufs=3) as outp:
            i1t = constp.tile([128, c1], i16)
            nc.sync.dma_start(out=i1t[:], in_=i1[:, :])
            i2t = constp.tile([128, c2], i16)
            nc.sync.dma_start(out=i2t[:], in_=i2[:, :])
            rt = constp.tile([BLK, nblk], fp32)
            nc.sync.dma_start(out=rt[:], in_=recip[:, :])
            zt = constp.tile([BLK, d], fp32)
            nc.vector.memset(zt[:], 0.0)
            nc.sync.dma_start(out=scr[dump:dump + BLK, :], in_=zt[:])

            stg = stag.tile([128, C * d], fp32)
            for s in range(NSEG):
                lo = s * SEG
                hi = min(n_nodes, lo + SEG)
                nc.gpsimd.dma_gather(
                    stg[:, s * cs * d:(s + 1) * cs * d].rearrange(
                        "p (c d) -> p c d", d=d),
                    feat[lo:hi, :],
                    i1t[:, s * (m_cap // 16):(s + 1) * (m_cap // 16)],
                    m_cap, m_cap, d,
                    single_packet=(m_cap <= 1024))
            # staging -> scratch in two halves so the copy overlaps phase 1
            half = (NSEG + 1) // 2 * cs
            for (a, b) in ((0, half), (half, C)):
                nc.sync.dma_start(
                    out=scr[a * BLK:b * BLK, :].rearrange(
                        "(c p) d -> p c d", p=BLK),
                    in_=stg[:, a * d:b * d].rearrange("p (c d) -> p c d", d=d))

            col = 0
            for nb in range(nblk):
                kj = k_sched[nb]
                sl = slice(nb * BLK, (nb + 1) * BLK)
                g = gp.tile([BLK, kj * d], fp32, tag="g")
                nc.gpsimd.dma_gather(
                    g[:].rearrange("p (c d) -> p c d", d=d),
                    scr[:, :],
                    i2t[:, col:col + kj * 8],
                    kj * BLK, kj * BLK, d,
                    single_packet=(kj * BLK <= 1024))
                col += kj * 8
                red = redp.tile([BLK, d], fp32)
                nc.vector.tensor_reduce(
                    out=red[:],
                    in_=g[:].rearrange("p (k d) -> p d k", d=d),
                    axis=mybir.AxisListType.X,
                    op=mybir.AluOpType.add,
                )
                o = outp.tile([BLK, d], fp32)
                nc.scalar.activation(
                    out=o[:],
                    in_=red[:],
                    func=mybir.ActivationFunctionType.Relu,
                    scale=rt[:, nb:nb + 1],
                )
                nc.sync.dma_start(out=out[sl, :], in_=o[:])

    from concourse import mybir as _mb
    _mb.codegen_inst_isa_subclasses(nc)
    _split_multi_waits(nc)
    return nc


def prep_core_inputs(features, neigh_idx, neigh_counts, n_cores=N_CORES):
    """Host-side sharding + index construction for the two-pass gather.

    Returns (in_maps, orders, params) where params = (m_cap, k_sched)."""
    n_nodes = features.shape[0]
    b = neigh_idx.shape[0]
    b_loc = b // n_cores
    nblk = b_loc // BLK

    idx = np.asarray(neigh_idx, dtype=np.int64)
    counts = np.asarray(neigh_counts, dtype=np.int64)
    feat = np.ascontiguousarray(np.asarray(features, dtype=np.float32))

    # --- global schedule (must be SPMD-uniform across cores) -------------
    per_core = []
    k_sched = np.ones(nblk, dtype=np.int64)
    m_need = 16                      # max unique rows in any (core, segment)
    for c in range(n_cores):
        sl = slice(c * b_loc, (c + 1) * b_loc)
        cnt_c = counts[sl]
        order = np.argsort(-cnt_c, kind="stable")
        sorted_cnt = cnt_c[order]
        sorted_idx = idx[sl][order]            # [b_loc, K]
        k_sched = np.maximum(k_sched,
                             sorted_cnt.reshape(nblk, BLK).max(axis=1))
        valid = (np.arange(idx.shape[1])[None, :] < sorted_cnt[:, None])
        rows = np.unique(sorted_idx[valid])    # sorted unique global rows
        segs = rows >> 15
        m_per_seg = np.bincount(segs, minlength=NSEG)
        m_need = max(m_need, int(m_per_seg.max()))
        per_core.append((order, sorted_cnt, sorted_idx, valid, rows, segs))

    m_cap = -(-m_need // 16) * 16
    k_sched = tuple(int(x) for x in k_sched)
    cs = -(-m_cap // 128)
    C = NSEG * cs
    dump = C * BLK
    assert C * BLK + BLK <= 32767, (m_cap, C)

    in_maps, orders = [], []
    for c in range(n_cores):
        order, sorted_cnt, sorted_idx, valid, rows, segs = per_core[c]
        # phase-1 index buffer: per segment, local uniques padded to m_cap
        # with a repeated valid index (reg must equal the non-neg count).
        seg_start = np.searchsorted(segs, np.arange(NSEG))
        seg_end = np.searchsorted(segs, np.arange(NSEG), side="right")
        i1_list = np.zeros((NSEG, m_cap), dtype=np.int16)
        for s in range(NSEG):
            ls = rows[seg_start[s]:seg_end[s]] - (s << 15)
            i1_list[s, :len(ls)] = ls.astype(np.int16)
            # padding stays 0 (= first row of the segment, always valid)
        i1_buf = _wrap16(i1_list.reshape(-1), NSEG * (m_cap // 16))

        # scratch position of each unique row
        j_in_seg = np.arange(len(rows)) - seg_start[segs]
        qpos = ((segs * cs + (j_in_seg >> 7)) << 7) + (j_in_seg & 127)

        # phase-2 index buffer: per block, slot-major positions (or dump)
        parts = []
        for nb in range(nblk):
            kj = k_sched[nb]
            nd = sorted_idx[nb * BLK:(nb + 1) * BLK, :kj]       # [128, kj]
            va = valid[nb * BLK:(nb + 1) * BLK, :kj]
            u = np.searchsorted(rows, nd)
            mat = np.where(va, qpos[np.minimum(u, len(rows) - 1)], dump)
            parts.append(mat.T.reshape(-1))     # slot-major: i = slot*128+p
        i2_buf = _wrap16(np.concatenate(parts),
                         sum(kj * 8 for kj in k_sched))

        rc = (1.0 / sorted_cnt.astype(np.float64)).astype(np.float32)
        recip_cols = np.ascontiguousarray(rc.reshape(nblk, BLK).T)

        orders.append(order)
        in_maps.append({
            "feat": feat,
            "i1": i1_buf,
            "i2": i2_buf,
            "recip": recip_cols,
        })
    return in_maps, orders, (m_cap, k_sched)


def kernel(features, neigh_idx, neigh_counts):
    from concourse.bass_utils import run_bass_kernel_spmd

    in_maps, orders, (m_cap, k_sched) = prep_core_inputs(
        features, neigh_idx, neigh_counts)
    key = ("nc2", N_NODES, BATCH // N_CORES, FEAT_DIM, m_cap, k_sched)
    if key not in _KERNEL_CACHE:
        _KERNEL_CACHE[key] = build_nc(m_cap, list(k_sched))
    nc = _KERNEL_CACHE[key]

    res = run_bass_kernel_spmd(nc, in_maps, list(range(N_CORES)))
    b_loc = BATCH // N_CORES
    out = np.empty((BATCH, FEAT_DIM), dtype=np.float32)
    for c in range(N_CORES):
        out_c = np.empty((b_loc, FEAT_DIM), dtype=np.float32)
        out_c[orders[c]] = res.results[c]["out"]
        out[c * b_loc:(c + 1) * b_loc] = out_c
    return out


# revision 8
# speedup vs baseline: 1.6430x; 1.6430x over previous
"""Trainium2 Bass kernel for nn_IntraAgg (GNN mean-neighbor aggregation).

reference:
    valid[b,k] = k < neigh_counts[b]
    out = relu( (sum_k valid[b,k] * features[neigh_idx[b,k]]) / neigh_counts[b] )

Strategy (8 NeuronCores, data-parallel over the batch):
  - shard neigh_idx/neigh_counts along B (8192 -> 1024 per core), replicate
    the features table.
  - host-side: cast indices to int32 and remap invalid (k >= count) slots to
    a zero row appended to the table (feat becomes [N+128, 64] with zero
    rows at the end); precompute 1/count as f32.  Gathering the zero row
    contributes nothing to the sum, so no bounds check and no per-block
    zero-fill are needed (the bounds check costs Q7 descriptor-generation
    cycles on every index, and Q7 emission is the kernel's bottleneck:
    ~11.6 ns/descriptor x 19.3K descriptors/core).
  - per core, per 128-node block: one indirect (gather) DMA pulls all
    128*32 neighbor rows into a [128, 32*64] SBUF tile.  A single strided
    DVE reduce sums over the 32 neighbor slots, then one ACT op applies
    relu(x * (1/count)) and the result is DMAed out.
"""

import numpy as np

N_NODES = 1_000_000
FEAT_DIM = 64
BATCH = 8192
MAX_NEIGH = 32
N_CORES = 8
BLK = 128  # nodes per block (SBUF partition dim)
ZPAD = 128  # zero rows appended to the feature table (dump target)

_KERNEL_CACHE = {}


def _split_multi_waits(nc):
    """walrus codegen accepts at most one sync-wait per instruction: hoist
    extra waits onto NoOp instructions inserted just before."""
    import bass_rust

    for fn in nc.m.functions:
        for bb in fn.blocks:
            new_list = []
            for inst in bb.instructions:
                si = inst.sync_info
                if si is not None and si.on_wait is not None and len(si.on_wait) > 1:
                    waits = list(si.on_wait)
                    for j, w in enumerate(waits[:-1]):
                        nop = bass_rust.InstNoOp(name=f"{inst.name}-sw{j}")
                        nop.engine = inst.engine
                        nop.sync_info = bass_rust.SyncInfo(on_wait=[w], on_update=[])
                        new_list.append(nop)
                    inst.sync_info = bass_rust.SyncInfo(
                        on_wait=[waits[-1]], on_update=list(si.on_update or [])
                    )
                new_list.append(inst)
            bb.instructions = new_list


def build_nc(n_nodes=N_NODES, b_loc=BATCH // N_CORES, k=MAX_NEIGH, d=FEAT_DIM,
             legalize=True, k_sched=None):
    """Build the per-core Bass program (SPMD: same program on every core)."""
    from concourse import bass, mybir
    from concourse.tile import TileContext

    assert b_loc % BLK == 0
    nblk = b_loc // BLK
    if k_sched is None:
        k_sched = [k] * nblk
    assert len(k_sched) == nblk and all(1 <= kj <= k for kj in k_sched)

    nc = bass.Bass()
    feat = nc.declare_dram_parameter("feat", [n_nodes + ZPAD, d],
                                     mybir.dt.float32, isOutput=False)
    idx = nc.declare_dram_parameter("idx", [b_loc, k], mybir.dt.int32,
                                    isOutput=False)
    recip = nc.declare_dram_parameter("recip", [b_loc, 1], mybir.dt.float32,
                                      isOutput=False)
    out = nc.declare_dram_parameter("out", [b_loc, d], mybir.dt.float32,
                                    isOutput=True)

    fp32 = mybir.dt.float32
    with TileContext(nc) as tc:
        with tc.tile_pool(name="idxp", bufs=5) as idxp, \
             tc.tile_pool(name="recp", bufs=5) as recp, \
             tc.tile_pool(name="gp", bufs=4) as gp, \
             tc.tile_pool(name="redp", bufs=3) as redp, \
             tc.tile_pool(name="outp", bufs=3) as outp:
            for b in range(nblk):
                kj = k_sched[b]
                sl = slice(b * BLK, (b + 1) * BLK)
                it = idxp.tile([BLK, k], mybir.dt.int32, tag="it")
                nc.sync.dma_start(out=it[:, :kj], in_=idx[sl, :kj])
                rt = recp.tile([BLK, 1], fp32)
                nc.sync.dma_start(out=rt[:], in_=recip[sl, :])

                g = gp.tile([BLK, k * d], fp32, tag="g")
                # HW consumes ONE index per partition per indirect DMA, so
                # gather neighbor k for all 128 nodes in one DMA; nodes are
                # count-sorted on the host so block b only needs k_sched[b]
                # DMAs.  Invalid slots point at the appended zero rows, so
                # every slot is written (no zero-fill, no bounds check).
                for kk in range(kj):
                    nc.gpsimd.indirect_dma_start(
                        out=g[:, kk * d:(kk + 1) * d],
                        out_offset=None,
                        in_=feat[:, :],
                        in_offset=bass.IndirectOffsetOnAxis(
                            ap=it[:, kk:kk + 1], axis=0),
                    )
                red = redp.tile([BLK, d], fp32)
                nc.vector.tensor_reduce(
                    out=red[:],
                    in_=g[:, :kj * d].rearrange("p (k d) -> p d k", d=d),
                    axis=mybir.AxisListType.X,
                    op=mybir.AluOpType.add,
                )
                o = outp.tile([BLK, d], fp32)
                nc.scalar.activation(
                    out=o[:],
                    in_=red[:],
                    func=mybir.ActivationFunctionType.Relu,
                    scale=rt[:, :1],
                )
                nc.sync.dma_start(out=out[sl, :], in_=o[:])

    if legalize:
        _split_multi_waits(nc)
    return nc


def prep_core_inputs(features, neigh_idx, neigh_counts, n_cores=N_CORES):
    """Host-side sharding/remapping.  Nodes are sorted by descending neighbor
    count within each core so later blocks need fewer gather DMAs.

    Returns (in_maps, orders, k_sched): per-core input dicts, per-core node
    permutations (sorted -> original via out[order] = out_sorted), and the
    per-block gather-DMA counts (max over cores)."""
    n_nodes = features.shape[0]
    b = neigh_idx.shape[0]
    b_loc = b // n_cores
    k = neigh_idx.shape[1]
    nblk = b_loc // BLK

    idx32 = np.asarray(neigh_idx, dtype=np.int32).copy()
    counts = np.asarray(neigh_counts, dtype=np.int64)
    recip = (1.0 / counts.astype(np.float64)).astype(np.float32)[:, None]

    feat = np.ascontiguousarray(np.concatenate(
        [np.asarray(features, dtype=np.float32),
         np.zeros((ZPAD, FEAT_DIM), dtype=np.float32)], axis=0))
    # invalid (k >= count) slots read an appended zero row; spread over the
    # ZPAD rows by the node's partition position so a block's dump reads
    # don't all hit one address.
    in_maps, orders = [], []
    k_sched = np.ones(nblk, dtype=np.int64)
    for c in range(n_cores):
        sl = slice(c * b_loc, (c + 1) * b_loc)
        cnt_c = counts[sl]
        order = np.argsort(-cnt_c, kind="stable")
        sorted_cnt = cnt_c[order]
        k_sched = np.maximum(
            k_sched, sorted_cnt.reshape(nblk, BLK).max(axis=1))
        sorted_idx = idx32[sl][order]
        invalid = (np.arange(k, dtype=np.int64)[None, :]
                   >= sorted_cnt[:, None])
        dump = (n_nodes + (np.arange(b_loc) % ZPAD)).astype(np.int32)
        sorted_idx = np.where(invalid, dump[:, None], sorted_idx)
        orders.append(order)
        in_maps.append({
            "feat": feat,
            "idx": np.ascontiguousarray(sorted_idx),
            "recip": np.ascontiguousarray(recip[sl][order]),
        })
    return in_maps, orders, tuple(int(x) for x in k_sched)


def _cache_key(k_sched):
    return ("nc", N_NODES, BATCH // N_CORES, MAX_NEIGH, FEAT_DIM, ZPAD,
            tuple(k_sched))


def kernel(features, neigh_idx, neigh_counts):
    from concourse.bass_utils import run_bass_kernel_spmd

    in_maps, orders, k_sched = prep_core_inputs(
        features, neigh_idx, neigh_counts)
    key = _cache_key(k_sched)
    if key not in _KERNEL_CACHE:
        _KERNEL_CACHE[key] = build_nc(k_sched=list(k_sched))
    nc = _KERNEL_CACHE[key]

    res = run_bass_kernel_spmd(nc, in_maps, list(range(N_CORES)))
    b_loc = BATCH // N_CORES
    out = np.empty((BATCH, FEAT_DIM), dtype=np.float32)
    for c in range(N_CORES):
        out_c = np.empty((b_loc, FEAT_DIM), dtype=np.float32)
        out_c[orders[c]] = res.results[c]["out"]
        out[c * b_loc:(c + 1) * b_loc] = out_c
    return out

